# revision 2
# baseline (speedup 1.0000x reference)
"""Trainium2 Bass kernel for CompositionalPhoneticsModel (segment_reduce).

Computation (reference):
    phone   = einsum('bth,hp->btp', enc_output, feature2phone) / sqrt(H)
    allo    = where(mapping>0, phone[:,:,None,:]*mapping, -inf)   # mapping is 0/1
    phoneme = max(allo, axis=-1)                                  # masked segment max
    out     = log_softmax(phoneme, axis=2)

Device strategy (8 NeuronCores, data-parallel over the B*T=8192 rows):
  * Host gathers feature2phone columns into segment-contiguous order
    (phones in 2 segments get duplicated columns; segments padded to even
    length -> NNZ=506, 5 length groups), folds in the 1/sqrt(H) scale, and
    pre-interleaves enc as [128, rows, NH] bf16. Device phoneme order is a
    permutation; the host un-permutes output columns at the end.
  * Pipeline: 8 row-blocks of 128 rows per core. All input DMAs are issued
    from the Sync engine's HWDGE queue in consumption order (enc block 0,
    then wk chunk by chunk, then the remaining enc blocks) so the first
    matmul can start as soon as ~290 KB have landed, not after all inputs.
  * PE: per block, 5 accumulating matmuls (K=128 each) into one PSUM bank.
    A short warmup matmul burst ramps the PE clock out of its half-rate
    power state while the first DMAs are still in flight.
  * Post-processing per pair of blocks: segment max = 5 strided reduce_max
    on DVE (only engine with a PSUM port that can max); exp with the
    activation accumulator (ScalarE, one op per block gives sum-exp free);
    ln on ScalarE; x - ln(sum) on GpSimd (SBUF only); output DMA on Sync.
    An early 1-element dummy Exp forces the joint Exp/Ln activation table
    to load during the DMA fill instead of stalling the first real exp.
"""

from contextlib import ExitStack

import numpy as np
import ml_dtypes

import concourse.bass as bass
import concourse.bacc as bacc
import concourse.tile as tile
from concourse import mybir
from concourse.bass_utils import run_bass_kernel_spmd

B, T, H = 8, 1024, 640
N_PHONEME, N_PHONE = 96, 230
N_CORES = 8
ROWS = B * T
RC = ROWS // N_CORES          # rows per core
NH = H // 128                 # contraction chunks
NB = RC // 128                # 128-row blocks per core
NP = NB // 2                  # block pairs
BF16 = ml_dtypes.bfloat16

WARMUP_N = 6                  # warmup matmuls (PE clock ramp)
WARMUP_W = 192                # columns per warmup matmul


def _structure(mapping: np.ndarray):
    """Segment-contiguous gather order, grouped by segment length (desc).

    Returns (col_ids, groups, perm):
      col_ids: phone index feeding each device matmul column (len NNZ)
      groups:  list of (L, nL, col_off, out_off) — nL segments of length L
               occupy matmul cols [col_off, col_off+nL*L) and device output
               cols [out_off, out_off+nL)
      perm:    perm[j] = original phoneme id of device output column j
    """
    segs = [np.nonzero(mapping[m] > 0)[0] for m in range(N_PHONEME)]
    assert min(len(s) for s in segs) >= 1
    # pad segment lengths up to even targets (repeating a member doesn't
    # change the max): fewer distinct lengths -> fewer DVE reduce ops.
    # Only worthwhile while the matmul width stays within one PSUM bank.
    padded = []
    for s in segs:
        t = ((len(s) + 1) // 2) * 2
        padded.append(np.concatenate([s, np.full(t - len(s), s[0], s.dtype)]))
    if sum(len(s) for s in padded) <= 512:
        segs = padded
    lengths = np.array([len(s) for s in segs])
    order = np.argsort(-lengths, kind="stable")
    col_ids, groups, perm = [], [], []
    i = 0
    while i < N_PHONEME:
        L = int(lengths[order[i]])
        j = i
        while j < N_PHONEME and lengths[order[j]] == L:
            j += 1
        groups.append((L, j - i, len(col_ids), i))
        for k in range(i, j):
            m = int(order[k])
            col_ids.extend(segs[m].tolist())
            perm.append(m)
        i = j
    return np.array(col_ids, dtype=np.int64), groups, np.array(perm, dtype=np.int64)


def _patch_act_tables():
    """Make Exp and Ln resolve to the same activation-table set.

    bacc's insert_act_table_loads models a single table slot, so a kernel
    alternating Exp/Ln reloads a 1.3us table on every transition.  act_info
    has a joint set ('natural_log_exp_and_others') containing both; keep the
    set list's order/indices intact but strip Exp/Ln from the other sets so
    the pass picks the joint set for both and emits a single load.
    """
    if getattr(bacc, "_act_tables_patched", False):
        return
    from concourse import hw_specs
    orig = hw_specs.get_activation_tables
    act = mybir.ActivationFunctionType

    def patched(module_arch):
        tabs = orig(module_arch)
        joint = [k for k, v in tabs.items() if act.Exp in v and act.Ln in v]
        if not joint:
            return tabs
        j = joint[0]
        return {
            k: (v if k == j else (v - {act.Exp, act.Ln}))
            for k, v in tabs.items()
        }

    bacc.get_activation_tables = patched
    bacc._act_tables_patched = True


def _build_program(nnz: int, groups):
    """Build + compile the per-core Bass program. Returns the Bacc object."""
    _patch_act_tables()
    nc = bacc.Bacc("TRN2", target_bir_lowering=False, debug=False)
    dt = mybir.dt
    act = mybir.ActivationFunctionType

    # enc interleaved: [128, RC, NH]; element (p, r, c) = enc[r, c*128+p]
    enck_d = nc.dram_tensor("enck", [128, RC, NH], dt.bfloat16, kind="ExternalInput")
    # W interleaved: [128, NH, nnz]; element (p, c, n) = W[c*128+p, n]
    wk_d = nc.dram_tensor("wk", [128, NH, nnz], dt.bfloat16, kind="ExternalInput")
    # out packed: [128, NB, 96]; element (p, b, m) = out[b*128+p, m]
    out_d = nc.dram_tensor("out", [128, NB, N_PHONEME], dt.float32, kind="ExternalOutput")

    with ExitStack() as ctx:
        tc = ctx.enter_context(tile.TileContext(nc))
        wpool = ctx.enter_context(tc.tile_pool(name="wpool", bufs=1))
        epool = ctx.enter_context(tc.tile_pool(name="epool", bufs=5))
        ppool = ctx.enter_context(tc.tile_pool(name="ppool", bufs=4, space="PSUM"))
        spool = ctx.enter_context(tc.tile_pool(name="spool", bufs=4))

        # scratch for warmup matmuls + dummy activation
        wu = wpool.tile([128, WARMUP_W], dt.bfloat16)
        nc.vector.memset(wu[:], 0.0)
        dumm = wpool.tile([128, 1], dt.float32)
        nc.scalar.activation(dumm[:], wu[:, 0:1], act.Exp)

        # --- input DMAs, all on the Sync HWDGE queue, consumption order ---
        # enc blocks: 0 | 1 | 23 | 45 | 67 (first block alone so the PE can
        # start earliest; pairs after that to save issue slots)
        eb = [1, 1, 2, 2, 2]
        et = []
        roff = 0
        for nblk in eb:
            t = epool.tile([128, nblk * 128, NH], dt.bfloat16)
            et.append((roff, nblk, t))
            roff += nblk * 128
        # wk chunk tiles: c=0 | c=1 | c=2..4
        wt0 = wpool.tile([128, 1, nnz], dt.bfloat16)
        wt1 = wpool.tile([128, 1, nnz], dt.bfloat16)
        wt234 = wpool.tile([128, 3, nnz], dt.bfloat16)

        r0, n0, t0 = et[0]
        nc.sync.dma_start(t0[:], enck_d[:, r0:r0 + n0 * 128, :])
        nc.sync.dma_start(wt0[:], wk_d[:, 0:1, :])
        nc.sync.dma_start(wt1[:], wk_d[:, 1:2, :])
        nc.sync.dma_start(wt234[:], wk_d[:, 2:5, :])
        for roff, nblk, t in et[1:]:
            nc.sync.dma_start(t[:], enck_d[:, roff:roff + nblk * 128, :])

        def wt_for(c):
            return wt0[:, 0, :] if c == 0 else (
                wt1[:, 0, :] if c == 1 else wt234[:, c - 2, :])

        def et_for(b):
            for roff, nblk, t in et:
                if roff <= b * 128 < roff + nblk * 128:
                    i = b - roff // 128
                    return t[:, i * 128:(i + 1) * 128, :]
            raise AssertionError

        # --- PE warmup: ramp the clock while DMAs land ---
        ps0 = ppool.tile([128, 2, 512], dt.float32, tag="ps")
        for _ in range(WARMUP_N):
            nc.tensor.matmul(ps0[:, 0, :WARMUP_W], wu[:, :128], wu[:],
                             start=True, stop=True)

        # --- main pipeline, one pair of 128-row blocks at a time ---
        for p in range(NP):
            ps = ps0 if p == 0 else ppool.tile([128, 2, 512], dt.float32, tag="ps")
            for r in range(2):
                b = 2 * p + r
                lhs = et_for(b)
                for c in range(NH):
                    nc.tensor.matmul(
                        ps[:, r, :nnz],
                        lhs[:, :, c],
                        wt_for(c),
                        start=(c == 0),
                        stop=(c == NH - 1),
                    )

            # segment max: one strided reduce per length group, batched over
            # both blocks of the pair (4D input AP [128, 2, nL, L])
            pmax = spool.tile([128, 2, N_PHONEME], dt.float32, tag="pmax")
            for (L, nL, coff, ooff) in groups:
                src = ps[:, :, coff:coff + nL * L].rearrange(
                    "p r (s l) -> p r s l", l=L
                )
                nc.vector.reduce_max(
                    pmax[:, :, ooff:ooff + nL], src, axis=mybir.AxisListType.X
                )

            # exp (no max-subtraction needed: |phone| <~ 9, exp fits fp32);
            # row-sum comes free via the activation accumulator
            ex = spool.tile([128, 2, N_PHONEME], dt.float32, tag="ex")
            se = spool.tile([128, 2], dt.float32, tag="se")
            for r in range(2):
                nc.scalar.activation(ex[:, r, :], pmax[:, r, :], act.Exp,
                                     accum_out=se[:, r:r + 1])
            lse = spool.tile([128, 2], dt.float32, tag="lse")
            nc.scalar.activation(lse[:], se[:], act.Ln)
            ott = spool.tile([128, 2, N_PHONEME], dt.float32, tag="ott")
            for r in range(2):
                nc.gpsimd.tensor_scalar_sub(
                    ott[:, r, :], pmax[:, r, :], lse[:, r:r + 1]
                )
            nc.sync.dma_start(out_d[:, 2 * p:2 * p + 2, :], ott[:])

    nc.compile()
    return nc


_CACHE: dict = {}


def _get_compiled(mapping: np.ndarray):
    key = mapping.astype(np.float32).tobytes()
    if _CACHE.get("key") != key:
        col_ids, groups, perm = _structure(mapping)
        nc = _build_program(len(col_ids), groups)
        _CACHE.update(key=key, col_ids=col_ids, groups=groups, perm=perm, nc=nc)
    return _CACHE["nc"], _CACHE["col_ids"], _CACHE["perm"]


def _prep_in_maps(enc_output, feature2phone, col_ids):
    scale = np.float32(1.0) / np.sqrt(np.float32(H))
    wg = (feature2phone.astype(np.float32) * scale)[:, col_ids].astype(BF16)
    # [H, nnz] -> [128, NH, nnz]
    wk = np.ascontiguousarray(wg.reshape(NH, 128, -1).transpose(1, 0, 2))
    # enc [ROWS, H] -> [128, ROWS, NH]
    e3 = enc_output.astype(BF16).reshape(ROWS, NH, 128)
    enck = np.ascontiguousarray(e3.transpose(2, 0, 1))
    in_maps = []
    for c in range(N_CORES):
        in_maps.append({
            "enck": np.ascontiguousarray(enck[:, c * RC:(c + 1) * RC, :]),
            "wk": wk,
        })
    return in_maps


def run_device(enc_output, feature2phone, mapping, trace=False, **kw):
    """Build/compile (cached), run on the 8 cores, return (output, BassKernelResults)."""
    enc_output = np.asarray(enc_output)
    feature2phone = np.asarray(feature2phone)
    mapping = np.asarray(mapping)
    nc, col_ids, perm = _get_compiled(mapping)
    in_maps = _prep_in_maps(enc_output, feature2phone, col_ids)
    res = run_bass_kernel_spmd(
        nc, in_maps, core_ids=list(range(N_CORES)), trace=trace, **kw
    )
    # device out [128, NB, 96] packed -> rows b*128+p
    dev = np.concatenate(
        [res.results[c]["out"].transpose(1, 0, 2).reshape(RC, N_PHONEME)
         for c in range(N_CORES)],
        axis=0,
    )
    out = np.empty_like(dev)
    out[:, perm] = dev
    return out.reshape(B, T, N_PHONEME).astype(np.float32), res


def kernel(enc_output, feature2phone, mapping):
    out, _ = run_device(enc_output, feature2phone, mapping)
    return out


# revision 7
# speedup vs baseline: 1.1383x; 1.1383x over previous
"""Trainium2 Bass kernel for CompositionalPhoneticsModel (segment_reduce).

Computation (reference):
    phone   = einsum('bth,hp->btp', enc_output, feature2phone) / sqrt(H)
    allo    = where(mapping>0, phone[:,:,None,:]*mapping, -inf)   # mapping is 0/1
    phoneme = max(allo, axis=-1)                                  # masked segment max
    out     = log_softmax(phoneme, axis=2)

Device strategy (8 NeuronCores, data-parallel over the B*T=8192 rows):
  * Host gathers feature2phone columns into segment-contiguous order
    (phones in 2 segments get duplicated columns; segments padded to even
    length -> NNZ=506, 5 length groups), folds in the 1/sqrt(H) scale, and
    pre-interleaves enc as [128, rows, NH] bf16. Device phoneme order is a
    permutation; the host un-permutes output columns at the end.
  * Pipeline: 8 row-blocks of 128 rows per core. All input DMAs are issued
    from the Sync engine's HWDGE queue in consumption order (enc block 0,
    then wk chunk by chunk, then the remaining enc blocks) so the first
    matmul can start as soon as ~290 KB have landed, not after all inputs.
  * PE: per block, 5 accumulating matmuls (K=128 each) into one PSUM bank.
    A short warmup matmul burst ramps the PE clock out of its half-rate
    power state while the first DMAs are still in flight.
  * Post-processing per pair of blocks: segment max = 5 strided reduce_max
    on DVE (only engine with a PSUM port that can max); exp with the
    activation accumulator (ScalarE, one op per block gives sum-exp free);
    ln on ScalarE; x - ln(sum) on GpSimd (SBUF only); output DMA on Sync.
    An early 1-element dummy Exp forces the joint Exp/Ln activation table
    to load during the DMA fill instead of stalling the first real exp.
"""

from contextlib import ExitStack

import numpy as np
import ml_dtypes

import concourse.bass as bass
import concourse.bacc as bacc
import concourse.tile as tile
from concourse import mybir
from concourse.bass_utils import run_bass_kernel_spmd

B, T, H = 8, 1024, 640
N_PHONEME, N_PHONE = 96, 230
N_CORES = 8
ROWS = B * T
RC = ROWS // N_CORES          # rows per core
NH = H // 128                 # contraction chunks
NB = RC // 128                # 128-row blocks per core
NP = NB // 2                  # block pairs
BF16 = ml_dtypes.bfloat16

WARMUP_N = 8                  # warmup matmuls (PE clock ramp)
WARMUP_W = 512                # columns per warmup matmul


def _structure(mapping: np.ndarray):
    """Segment-contiguous gather order, grouped by segment length (desc).

    Returns (col_ids, groups, perm):
      col_ids: phone index feeding each device matmul column (len NNZ)
      groups:  list of (L, nL, col_off, out_off) — nL segments of length L
               occupy matmul cols [col_off, col_off+nL*L) and device output
               cols [out_off, out_off+nL)
      perm:    perm[j] = original phoneme id of device output column j
    """
    segs = [np.nonzero(mapping[m] > 0)[0] for m in range(N_PHONEME)]
    assert min(len(s) for s in segs) >= 1
    # pad segment lengths up to even targets (repeating a member doesn't
    # change the max): fewer distinct lengths -> fewer DVE reduce ops.
    # Only worthwhile while the matmul width stays within one PSUM bank.
    padded = []
    for s in segs:
        t = ((len(s) + 1) // 2) * 2
        padded.append(np.concatenate([s, np.full(t - len(s), s[0], s.dtype)]))
    if sum(len(s) for s in padded) <= 512:
        segs = padded
    lengths = np.array([len(s) for s in segs])
    order = np.argsort(-lengths, kind="stable")
    col_ids, groups, perm = [], [], []
    i = 0
    while i < N_PHONEME:
        L = int(lengths[order[i]])
        j = i
        while j < N_PHONEME and lengths[order[j]] == L:
            j += 1
        groups.append((L, j - i, len(col_ids), i))
        for k in range(i, j):
            m = int(order[k])
            col_ids.extend(segs[m].tolist())
            perm.append(m)
        i = j
    return np.array(col_ids, dtype=np.int64), groups, np.array(perm, dtype=np.int64)


def _patch_act_tables():
    """Make Exp and Ln resolve to the same activation-table set.

    bacc's insert_act_table_loads models a single table slot, so a kernel
    alternating Exp/Ln reloads a 1.3us table on every transition.  act_info
    has a joint set ('natural_log_exp_and_others') containing both; keep the
    set list's order/indices intact but strip Exp/Ln from the other sets so
    the pass picks the joint set for both and emits a single load.
    """
    if getattr(bacc, "_act_tables_patched", False):
        return
    from concourse import hw_specs
    orig = hw_specs.get_activation_tables
    act = mybir.ActivationFunctionType

    def patched(module_arch):
        tabs = orig(module_arch)
        joint = [k for k, v in tabs.items() if act.Exp in v and act.Ln in v]
        if not joint:
            return tabs
        j = joint[0]
        return {
            k: (v if k == j else (v - {act.Exp, act.Ln}))
            for k, v in tabs.items()
        }

    bacc.get_activation_tables = patched
    bacc._act_tables_patched = True


def _build_program(nnz: int, groups):
    """Build + compile the per-core Bass program. Returns the Bacc object."""
    _patch_act_tables()
    nc = bacc.Bacc("TRN2", target_bir_lowering=False, debug=False)
    dt = mybir.dt
    act = mybir.ActivationFunctionType

    # enc interleaved: [128, RC, NH]; element (p, r, c) = enc[r, c*128+p]
    enck_d = nc.dram_tensor("enck", [128, RC, NH], dt.bfloat16, kind="ExternalInput")
    # W interleaved: [128, NH, nnz]; element (p, c, n) = W[c*128+p, n]
    wk_d = nc.dram_tensor("wk", [128, NH, nnz], dt.bfloat16, kind="ExternalInput")
    # out packed: [128, NB, 96]; element (p, b, m) = out[b*128+p, m]
    out_d = nc.dram_tensor("out", [128, NB, N_PHONEME], dt.float32, kind="ExternalOutput")

    with ExitStack() as ctx:
        tc = ctx.enter_context(tile.TileContext(nc))
        wpool = ctx.enter_context(tc.tile_pool(name="wpool", bufs=1))
        epool = ctx.enter_context(tc.tile_pool(name="epool", bufs=5))
        ppool = ctx.enter_context(tc.tile_pool(name="ppool", bufs=4, space="PSUM"))
        spool = ctx.enter_context(tc.tile_pool(name="spool", bufs=4))

        # scratch for warmup matmuls + dummy activation
        wu = wpool.tile([128, WARMUP_W], dt.bfloat16)
        nc.vector.memset(wu[:], 0.0)

        # enc blocks: 0 | 1 | 23 | 45 | 67 (first block alone so the PE can
        # start earliest; pairs after that to save issue slots)
        eb = [1, 1, 2, 2, 2]
        et = []
        roff = 0
        for nblk in eb:
            t = epool.tile([128, nblk * 128, NH], dt.bfloat16)
            et.append((roff, nblk, t))
            roff += nblk * 128
        # wk chunk tiles: c=0..1 | c=2..4
        wt01 = wpool.tile([128, 2, nnz], dt.bfloat16)
        wt234 = wpool.tile([128, 3, nnz], dt.bfloat16)

        # enc block 0 goes out on the Scalar engine's HWDGE queue (issued
        # before the act-table load lands on that engine) so it overlaps the
        # wk stream on the Sync queue; everything else is consumption-ordered
        # on Sync.
        r0, n0, t0 = et[0]
        nc.scalar.dma_start(t0[:], enck_d[:, r0:r0 + n0 * 128, :])
        dumm = wpool.tile([128, 1], dt.float32)
        nc.scalar.activation(dumm[:], wu[:, 0:1], act.Exp)

        nc.sync.dma_start(wt01[:], wk_d[:, 0:2, :])
        nc.sync.dma_start(wt234[:], wk_d[:, 2:5, :])
        for roff, nblk, t in et[1:]:
            nc.sync.dma_start(t[:], enck_d[:, roff:roff + nblk * 128, :])

        def wt_for(c):
            return wt01[:, c, :] if c < 2 else wt234[:, c - 2, :]

        def et_for(b):
            for roff, nblk, t in et:
                if roff <= b * 128 < roff + nblk * 128:
                    i = b - roff // 128
                    return t[:, i * 128:(i + 1) * 128, :]
            raise AssertionError

        # --- PE warmup: ramp the clock while DMAs land ---
        ps0 = ppool.tile([128, 2, 512], dt.float32, tag="ps")
        for _ in range(WARMUP_N):
            nc.tensor.matmul(ps0[:, 0, :WARMUP_W], wu[:, :128], wu[:],
                             start=True, stop=True)

        # --- main pipeline ---
        # Pairs of 128-row blocks share one PSUM tile (2 banks) and one DVE
        # reduce set; the last two blocks are processed singly so block 6's
        # post-chain overlaps block 7's matmuls and block 7's tail is minimal.
        # Subs go to GpSimd mid-stream (slow but fully hidden) and to the
        # then-idle DVE for the final blocks.
        def matmuls(ps, r, b):
            lhs = et_for(b)
            for c in range(NH):
                nc.tensor.matmul(
                    ps[:, r, :nnz],
                    lhs[:, :, c],
                    wt_for(c),
                    start=(c == 0),
                    stop=(c == NH - 1),
                )

        def seg_max(ps, pmax, nr):
            # nr row blocks: strided reduce per length group, 4D AP
            for (L, nL, coff, ooff) in groups:
                src = ps[:, :nr, coff:coff + nL * L].rearrange(
                    "p r (s l) -> p r s l", l=L
                )
                nc.vector.reduce_max(
                    pmax[:, :nr, ooff:ooff + nL], src, axis=mybir.AxisListType.X
                )

        def softmax_tail(pmax, ott, r, sub_engine):
            # exp with accumulator -> se; ln; out = pmax - lse
            ex = spool.tile([128, N_PHONEME], dt.float32, tag="ex")
            se = spool.tile([128, 1], dt.float32, tag="se")
            nc.scalar.activation(ex[:], pmax[:, r, :], act.Exp,
                                 accum_out=se[:])
            lse = spool.tile([128, 1], dt.float32, tag="lse")
            nc.scalar.activation(lse[:], se[:], act.Ln)
            sub_engine.tensor_scalar_sub(ott[:, r, :], pmax[:, r, :], lse[:])

        for p in range(NP - 1):
            ps = ps0 if p == 0 else ppool.tile([128, 2, 512], dt.float32, tag="ps")
            for r in range(2):
                matmuls(ps, r, 2 * p + r)
            pmax = spool.tile([128, 2, N_PHONEME], dt.float32, tag="pmax")
            seg_max(ps, pmax, 2)
            ott = spool.tile([128, 2, N_PHONEME], dt.float32, tag="ott")
            for r in range(2):
                softmax_tail(pmax, ott, r, nc.gpsimd)
            nc.sync.dma_start(out_d[:, 2 * p:2 * p + 2, :], ott[:])

        # last two blocks, singly
        ps = ppool.tile([128, 2, 512], dt.float32, tag="ps")
        for r, b in ((0, NB - 2), (1, NB - 1)):
            matmuls(ps, r, b)
            pmax = spool.tile([128, 1, N_PHONEME], dt.float32, tag=f"pmaxl{r}")
            seg_max(ps[:, r:r + 1, :], pmax, 1)
            ott = spool.tile([128, 1, N_PHONEME], dt.float32, tag=f"ottl{r}")
            softmax_tail(pmax, ott, 0, nc.vector)
            nc.sync.dma_start(out_d[:, b:b + 1, :], ott[:])

    nc.compile()
    return nc


_CACHE: dict = {}


def _get_compiled(mapping: np.ndarray):
    key = mapping.astype(np.float32).tobytes()
    if _CACHE.get("key") != key:
        col_ids, groups, perm = _structure(mapping)
        nc = _build_program(len(col_ids), groups)
        _CACHE.update(key=key, col_ids=col_ids, groups=groups, perm=perm, nc=nc)
    return _CACHE["nc"], _CACHE["col_ids"], _CACHE["perm"]


def _prep_in_maps(enc_output, feature2phone, col_ids):
    scale = np.float32(1.0) / np.sqrt(np.float32(H))
    wg = (feature2phone.astype(np.float32) * scale)[:, col_ids].astype(BF16)
    # [H, nnz] -> [128, NH, nnz]
    wk = np.ascontiguousarray(wg.reshape(NH, 128, -1).transpose(1, 0, 2))
    # enc [ROWS, H] -> [128, ROWS, NH]
    e3 = enc_output.astype(BF16).reshape(ROWS, NH, 128)
    enck = np.ascontiguousarray(e3.transpose(2, 0, 1))
    in_maps = []
    for c in range(N_CORES):
        in_maps.append({
            "enck": np.ascontiguousarray(enck[:, c * RC:(c + 1) * RC, :]),
            "wk": wk,
        })
    return in_maps


def run_device(enc_output, feature2phone, mapping, trace=False, **kw):
    """Build/compile (cached), run on the 8 cores, return (output, BassKernelResults)."""
    enc_output = np.asarray(enc_output)
    feature2phone = np.asarray(feature2phone)
    mapping = np.asarray(mapping)
    nc, col_ids, perm = _get_compiled(mapping)
    in_maps = _prep_in_maps(enc_output, feature2phone, col_ids)
    res = run_bass_kernel_spmd(
        nc, in_maps, core_ids=list(range(N_CORES)), trace=trace, **kw
    )
    # device out [128, NB, 96] packed -> rows b*128+p
    dev = np.concatenate(
        [res.results[c]["out"].transpose(1, 0, 2).reshape(RC, N_PHONEME)
         for c in range(N_CORES)],
        axis=0,
    )
    out = np.empty_like(dev)
    out[:, perm] = dev
    return out.reshape(B, T, N_PHONEME).astype(np.float32), res


def kernel(enc_output, feature2phone, mapping):
    out, _ = run_device(enc_output, feature2phone, mapping)
    return out


# revision 15
# speedup vs baseline: 1.2713x; 1.1168x over previous
"""Trainium2 Bass kernel for CompositionalPhoneticsModel (segment_reduce).

Computation (reference):
    phone   = einsum('bth,hp->btp', enc_output, feature2phone) / sqrt(H)
    allo    = where(mapping>0, phone[:,:,None,:]*mapping, -inf)   # mapping is 0/1
    phoneme = max(allo, axis=-1)                                  # masked segment max
    out     = log_softmax(phoneme, axis=2)

Device strategy (8 NeuronCores, data-parallel over the B*T=8192 rows):
  * Host gathers feature2phone columns into segment-contiguous order
    (phones in 2 segments get duplicated columns; segments padded to even
    length -> NNZ=506, 5 length groups), folds in the 1/sqrt(H) scale, and
    pre-interleaves enc as [128, rows, NH] bf16. Device phoneme order is a
    permutation; the host un-permutes output columns at the end.
  * Pipeline: 8 row-blocks of 128 rows per core. All input DMAs are issued
    from the Sync engine's HWDGE queue in consumption order (enc block 0,
    then wk chunk by chunk, then the remaining enc blocks) so the first
    matmul can start as soon as ~290 KB have landed, not after all inputs.
  * PE: per block, 5 accumulating matmuls (K=128 each) into one PSUM bank.
    A short warmup matmul burst ramps the PE clock out of its half-rate
    power state while the first DMAs are still in flight.
  * Post-processing per pair of blocks: segment max = 5 strided reduce_max
    on DVE (only engine with a PSUM port that can max); exp with the
    activation accumulator (ScalarE, one op per block gives sum-exp free);
    ln on ScalarE; x - ln(sum) on GpSimd (SBUF only); output DMA on Sync.
    An early 1-element dummy Exp forces the joint Exp/Ln activation table
    to load during the DMA fill instead of stalling the first real exp.
"""

from contextlib import ExitStack

import numpy as np
import ml_dtypes

import concourse.bass as bass
import concourse.bacc as bacc
import concourse.tile as tile
from concourse import mybir
from concourse.bass_utils import run_bass_kernel_spmd

B, T, H = 8, 1024, 640
N_PHONEME, N_PHONE = 96, 230
N_CORES = 8
ROWS = B * T
RC = ROWS // N_CORES          # rows per core
NH = H // 128                 # contraction chunks
NB = RC // 128                # 128-row blocks per core
NP = NB // 2                  # block pairs
BF16 = ml_dtypes.bfloat16

WARMUP_N = 8                  # warmup matmuls (PE clock ramp)
WARMUP_W = 512                # columns per warmup matmul


def _structure(mapping: np.ndarray):
    """Segment-contiguous gather order, grouped by segment length (desc).

    Returns (col_ids, groups, perm):
      col_ids: phone index feeding each device matmul column (len NNZ)
      groups:  list of (L, nL, col_off, out_off) — nL segments of length L
               occupy matmul cols [col_off, col_off+nL*L) and device output
               cols [out_off, out_off+nL)
      perm:    perm[j] = original phoneme id of device output column j
    """
    segs = [np.nonzero(mapping[m] > 0)[0] for m in range(N_PHONEME)]
    assert min(len(s) for s in segs) >= 1
    # pad segment lengths up to even targets, and 2 up to 4 (repeating a
    # member doesn't change the max): fewer distinct lengths -> fewer DVE
    # reduce ops. Only worthwhile while the matmul width stays within one
    # PSUM bank (512 fp32).
    padded = []
    for s in segs:
        t = ((len(s) + 1) // 2) * 2
        if t == 2:
            t = 4
        padded.append(np.concatenate([s, np.full(t - len(s), s[0], s.dtype)]))
    if sum(len(s) for s in padded) <= 512:
        segs = padded
    lengths = np.array([len(s) for s in segs])
    order = np.argsort(-lengths, kind="stable")
    col_ids, groups, perm = [], [], []
    i = 0
    while i < N_PHONEME:
        L = int(lengths[order[i]])
        j = i
        while j < N_PHONEME and lengths[order[j]] == L:
            j += 1
        groups.append((L, j - i, len(col_ids), i))
        for k in range(i, j):
            m = int(order[k])
            col_ids.extend(segs[m].tolist())
            perm.append(m)
        i = j
    return np.array(col_ids, dtype=np.int64), groups, np.array(perm, dtype=np.int64)


def _patch_act_tables():
    """Make Exp and Ln resolve to the same activation-table set.

    bacc's insert_act_table_loads models a single table slot, so a kernel
    alternating Exp/Ln reloads a 1.3us table on every transition.  act_info
    has a joint set ('natural_log_exp_and_others') containing both; keep the
    set list's order/indices intact but strip Exp/Ln from the other sets so
    the pass picks the joint set for both and emits a single load.
    """
    if getattr(bacc, "_act_tables_patched", False):
        return
    from concourse import hw_specs
    orig = hw_specs.get_activation_tables
    act = mybir.ActivationFunctionType

    def patched(module_arch):
        tabs = orig(module_arch)
        joint = [k for k, v in tabs.items() if act.Exp in v and act.Ln in v]
        if not joint:
            return tabs
        j = joint[0]
        pin = {act.Exp, act.Ln, act.Identity, act.Copy} & tabs[j]
        return {
            k: (v if k == j else (v - pin))
            for k, v in tabs.items()
        }

    bacc.get_activation_tables = patched
    bacc._act_tables_patched = True


def _build_program(nnz: int, groups):
    """Build + compile the per-core Bass program. Returns the Bacc object."""
    _patch_act_tables()
    nc = bacc.Bacc("TRN2", target_bir_lowering=False, debug=False)
    dt = mybir.dt
    act = mybir.ActivationFunctionType

    # enc interleaved: [128, RC, NH]; element (p, r, c) = enc[r, c*128+p]
    enck_d = nc.dram_tensor("enck", [128, RC, NH], dt.bfloat16, kind="ExternalInput")
    # W interleaved: [128, NH, nnz]; element (p, c, n) = W[c*128+p, n]
    wk_d = nc.dram_tensor("wk", [128, NH, nnz], dt.bfloat16, kind="ExternalInput")
    # out packed: [128, NB, 96]; element (p, b, m) = out[b*128+p, m]
    out_d = nc.dram_tensor("out", [128, NB, N_PHONEME], dt.float32, kind="ExternalOutput")

    with ExitStack() as ctx:
        tc = ctx.enter_context(tile.TileContext(nc))
        wpool = ctx.enter_context(tc.tile_pool(name="wpool", bufs=1))
        epool = ctx.enter_context(tc.tile_pool(name="epool", bufs=5))
        ppool = ctx.enter_context(tc.tile_pool(name="ppool", bufs=2, space="PSUM"))
        lpool = ctx.enter_context(tc.tile_pool(name="lpool", bufs=1, space="PSUM"))
        spool = ctx.enter_context(tc.tile_pool(name="spool", bufs=4))

        # scratch for warmup matmuls + dummy activation
        wu = wpool.tile([128, WARMUP_W], dt.bfloat16)
        nc.vector.memset(wu[:], 0.0)

        # enc blocks: 0 | 1 | 23 | 45 | 67 (first block alone so the PE can
        # start earliest; pairs after that to save issue slots)
        eb = [1, 1, 2, 2, 2]
        et = []
        roff = 0
        for nblk in eb:
            t = epool.tile([128, nblk * 128, NH], dt.bfloat16)
            et.append((roff, nblk, t))
            roff += nblk * 128
        wt = wpool.tile([128, NH, nnz], dt.bfloat16)

        # enc block 0 goes out on the Scalar engine's HWDGE queue (issued
        # before the act-table load lands on that engine) so it overlaps the
        # wk stream on the Sync queue; everything else is consumption-ordered
        # on Sync. wk stays one transfer: the PE consumes it faster than HBM
        # can stream it, so chunk-split arrivals only add ramp-resetting
        # stalls inside block 0.
        r0, n0, t0 = et[0]
        nc.scalar.dma_start(t0[:], enck_d[:, r0:r0 + n0 * 128, :])
        dumm = wpool.tile([128, 1], dt.float32)
        nc.scalar.activation(dumm[:], wu[:, 0:1], act.Exp)

        nc.sync.dma_start(wt[:], wk_d[:])
        for roff, nblk, t in et[1:]:
            nc.sync.dma_start(t[:], enck_d[:, roff:roff + nblk * 128, :])

        def wt_for(c, lo, hi):
            return wt[:, c, lo:hi]

        def et_for(b):
            for roff, nblk, t in et:
                if roff <= b * 128 < roff + nblk * 128:
                    i = b - roff // 128
                    return t[:, i * 128:(i + 1) * 128, :]
            raise AssertionError

        # --- PE warmup: ramp the clock while DMAs land ---
        ps0 = ppool.tile([128, 2, 512], dt.float32, tag="ps")
        for _ in range(WARMUP_N):
            nc.tensor.matmul(ps0[:, 0, :WARMUP_W], wu[:, :128], wu[:],
                             start=True, stop=True)

        # --- main pipeline ---
        # Pairs of 128-row blocks share one PSUM tile (2 banks), one DVE
        # reduce set, one batched exp, and one out-DMA. Mid-stream subs run
        # on the Scalar engine as Identity-with-bias (Identity shares the
        # Exp/Ln activation table, so no reloads); the DVE keeps only
        # reduces and the row-sums. The last pair's matmuls are split
        # column-wise across two PSUM tiles so its heavy reduce groups start
        # while the narrow tail columns are still streaming, and its subs go
        # to the then-idle DVE.
        def matmuls(ps, r, b, lo, hi):
            lhs = et_for(b)
            for c in range(NH):
                nc.tensor.matmul(
                    ps[:, r, :hi - lo],
                    lhs[:, :, c],
                    wt_for(c, lo, hi),
                    start=(c == 0),
                    stop=(c == NH - 1),
                )

        def seg_max(ps, pmax, gset, base):
            # strided reduce per length group over both row blocks (4D AP)
            for (L, nL, coff, ooff) in gset:
                src = ps[:, :, coff - base:coff - base + nL * L].rearrange(
                    "p r (s l) -> p r s l", l=L
                )
                nc.vector.reduce_max(
                    pmax[:, :, ooff:ooff + nL], src, axis=mybir.AxisListType.X
                )

        def softmax_pair(pmax, ott, sub_dve):
            # exp (no max-subtraction needed: |phone| <~ 9, exp fits fp32),
            # batched over the pair; row-sums + ln; out = pmax - lse
            ex = spool.tile([128, 2, N_PHONEME], dt.float32, tag="ex")
            nc.scalar.activation(ex[:], pmax[:], act.Exp)
            se = spool.tile([128, 2], dt.float32, tag="se")
            nc.vector.reduce_sum(se[:], ex[:], axis=mybir.AxisListType.X)
            lse = spool.tile([128, 2], dt.float32, tag="lse")
            nc.scalar.activation(lse[:], se[:], act.Ln)
            if sub_dve:
                for r in range(2):
                    nc.vector.tensor_scalar_sub(
                        ott[:, r, :], pmax[:, r, :], lse[:, r:r + 1]
                    )
            else:
                nlse = spool.tile([128, 2], dt.float32, tag="nlse")
                nc.scalar.activation(nlse[:], lse[:], act.Identity, scale=-1.0)
                for r in range(2):
                    nc.scalar.activation(ott[:, r, :], pmax[:, r, :],
                                         act.Identity, bias=nlse[:, r:r + 1])

        for p in range(NP - 1):
            ps = ps0 if p == 0 else ppool.tile([128, 2, 512], dt.float32, tag="ps")
            for r in range(2):
                matmuls(ps, r, 2 * p + r, 0, nnz)
            pmax = spool.tile([128, 2, N_PHONEME], dt.float32, tag="pmax")
            seg_max(ps, pmax, groups, 0)
            ott = spool.tile([128, 2, N_PHONEME], dt.float32, tag="ott")
            softmax_pair(pmax, ott, sub_dve=False)
            nc.sync.dma_start(out_d[:, 2 * p:2 * p + 2, :], ott[:])

        # last pair: columns split across two PSUM tiles at a group boundary
        split = groups[-1][2]          # col offset of the last (L=4) group
        ps_a = lpool.tile([128, 2, 512], dt.float32, tag="psa")
        ps_b = lpool.tile([128, 2, 512], dt.float32, tag="psb")
        b0, b1 = NB - 2, NB - 1
        matmuls(ps_a, 0, b0, 0, split)
        matmuls(ps_b, 0, b0, split, nnz)
        matmuls(ps_a, 1, b1, 0, split)
        pmax = spool.tile([128, 2, N_PHONEME], dt.float32, tag="pmax")
        seg_max(ps_a, pmax, groups[:-1], 0)     # overlaps ps_b matmuls
        matmuls(ps_b, 1, b1, split, nnz)
        seg_max(ps_b, pmax, groups[-1:], split)
        ott = spool.tile([128, 2, N_PHONEME], dt.float32, tag="ott")
        softmax_pair(pmax, ott, sub_dve=True)
        nc.sync.dma_start(out_d[:, NB - 2:NB, :], ott[:])

    nc.compile()
    return nc


_CACHE: dict = {}


def _get_compiled(mapping: np.ndarray):
    key = mapping.astype(np.float32).tobytes()
    if _CACHE.get("key") != key:
        col_ids, groups, perm = _structure(mapping)
        nc = _build_program(len(col_ids), groups)
        _CACHE.update(key=key, col_ids=col_ids, groups=groups, perm=perm, nc=nc)
    return _CACHE["nc"], _CACHE["col_ids"], _CACHE["perm"]


def _prep_in_maps(enc_output, feature2phone, col_ids):
    scale = np.float32(1.0) / np.sqrt(np.float32(H))
    wg = (feature2phone.astype(np.float32) * scale)[:, col_ids].astype(BF16)
    # [H, nnz] -> [128, NH, nnz]
    wk = np.ascontiguousarray(wg.reshape(NH, 128, -1).transpose(1, 0, 2))
    # enc [ROWS, H] -> [128, ROWS, NH]
    e3 = enc_output.astype(BF16).reshape(ROWS, NH, 128)
    enck = np.ascontiguousarray(e3.transpose(2, 0, 1))
    in_maps = []
    for c in range(N_CORES):
        in_maps.append({
            "enck": np.ascontiguousarray(enck[:, c * RC:(c + 1) * RC, :]),
            "wk": wk,
        })
    return in_maps


def run_device(enc_output, feature2phone, mapping, trace=False, **kw):
    """Build/compile (cached), run on the 8 cores, return (output, BassKernelResults)."""
    enc_output = np.asarray(enc_output)
    feature2phone = np.asarray(feature2phone)
    mapping = np.asarray(mapping)
    nc, col_ids, perm = _get_compiled(mapping)
    in_maps = _prep_in_maps(enc_output, feature2phone, col_ids)
    res = run_bass_kernel_spmd(
        nc, in_maps, core_ids=list(range(N_CORES)), trace=trace, **kw
    )
    # device out [128, NB, 96] packed -> rows b*128+p
    dev = np.concatenate(
        [res.results[c]["out"].transpose(1, 0, 2).reshape(RC, N_PHONEME)
         for c in range(N_CORES)],
        axis=0,
    )
    out = np.empty_like(dev)
    out[:, perm] = dev
    return out.reshape(B, T, N_PHONEME).astype(np.float32), res


def kernel(enc_output, feature2phone, mapping):
    out, _ = run_device(enc_output, feature2phone, mapping)
    return out
